# revision 7
# baseline (speedup 1.0000x reference)
"""Distributed AQT int8 fake-quant matmul on 8 Trainium2 NeuronCores.

Computes reference:
    lhs_q = fake_quant_int8(lhs); rhs_q = fake_quant_int8(rhs)
    out = lhs_q @ rhs_q            # [4096, 8192] f32

Sharding: 2x4 core grid. Core (i,j) computes the [2048, 2048] output block
(M-half i, N-quarter j) as a K=2048 matmul.

Per the sharding hint, the per-tensor scale is replicated: the global absmax
scale (2 scalars) is computed on host in f32 (bit-identical to the reference
reduction, which is order-independent) and broadcast to all cores; each
device quantizes its shard locally. Set DEVICE_SCALES=True to instead
compute the absmax fully on-device (disjoint 1/8 slices per core + a [128,2]
AllReduce(max) collective) — same numerics to ~1ulp, but pays the ncfw
collective entry barrier (~80us on this runtime).

Quantized values (ints in [-127,127]) are exact in bf16, so the matmul runs
at full bf16 PE rate and the result matches the f32 fake-quant reference to
~1e-6. Rounding uses the magic-constant trick: bf16(f32(x*s + 1.5*2^23) - C)
== round-half-even(x*s), bit-identical to jnp.round.

Pipeline (per core): stream f32 shards once; quantize on ACT (x*s+C) + DVE
(-C, cast bf16) into persistent SBUF caches; 1024 bf16 matmuls in 16 waves
(one 128-row m-tile x full N=2048 per wave, 4 PSUM banks, one weight load
per 4 matmuls); dequantized PSUM evacuation on ACT; outputs DMAed via
gpsimd so input DMAs (sync) are never queued behind them.
"""

import numpy as np

import concourse.bass as bass
import concourse.bass_isa as bass_isa
import concourse.mybir as mybir
import concourse.tile as tile
from concourse import bacc
from concourse.bass_utils import run_bass_kernel_spmd

# Problem shape (hardcoded per contract)
M_FULL, K, N_FULL = 4096, 2048, 8192
RI, CJ = 2, 4                      # core grid: M shards x N shards
M, N = M_FULL // RI, N_FULL // CJ  # 2048 x 2048 per-core output block
P = 128
KT = K // P                        # 16 k-tiles
MT = M // P                        # 16 m-tiles (one wave each)
NB = N // 512                      # 4 n-blocks of 512
MSL_W = M // CJ                    # 512: per-core lhs max-slice width
MSR_W = N // RI                    # 1024: per-core rhs max-slice width
C_MAGIC = 12582912.0               # 1.5 * 2^23
CLIP = 127.0
NCORES = RI * CJ

F32 = mybir.dt.float32
BF16 = mybir.dt.bfloat16
AF = mybir.ActivationFunctionType

DEVICE_SCALES = False  # True: on-device absmax + AllReduce(max) collective

# tuning knobs
STN_BUFS = 4   # [P,2048] f32 input staging (rhs k-rows)
STM_BUFS = 6   # [P,512] f32 input staging (lhsT chunks)
OST_BUFS = 4   # [P,512] f32 output staging


def _build_nc(device_scales):
    nc = bacc.Bacc("TRN2", target_bir_lowering=False, debug=False,
                   num_devices=NCORES)
    lhsT = nc.dram_tensor("lhsT", [K, M], F32, kind="ExternalInput")
    rhs = nc.dram_tensor("rhs", [K, N], F32, kind="ExternalInput")
    if device_scales:
        msl = nc.dram_tensor("msl", [K, MSL_W], F32, kind="ExternalInput")
        msr = nc.dram_tensor("msr", [K, MSR_W], F32, kind="ExternalInput")
    else:
        msl = msr = None
        scales = nc.dram_tensor("scales", [P, 4], F32, kind="ExternalInput")
    out = nc.dram_tensor("out", [M, N], F32, kind="ExternalOutput")

    with tile.TileContext(nc) as tc:
        if device_scales:
            _emit(nc, tc, lhsT, rhs, out, msl=msl, msr=msr)
        else:
            _emit(nc, tc, lhsT, rhs, out, scales=scales)
    nc.compile()
    return nc


def _emit(nc, tc, lhsT, rhs, out, msl=None, msr=None, scales=None):
    from contextlib import ExitStack
    ctx = ExitStack()
    with ctx:
        pconst = ctx.enter_context(tc.tile_pool(name="const", bufs=1))
        pred = ctx.enter_context(tc.tile_pool(name="red", bufs=3))
        pstn = ctx.enter_context(tc.tile_pool(name="stn", bufs=STN_BUFS))
        pstm = ctx.enter_context(tc.tile_pool(name="stm", bufs=STM_BUFS))
        pcache = ctx.enter_context(tc.tile_pool(name="cache", bufs=1))
        ppsum = ctx.enter_context(tc.tile_pool(name="psum", bufs=8, space="PSUM"))
        post = ctx.enter_context(tc.tile_pool(name="ost", bufs=OST_BUFS))

        sL = pconst.tile([P, 1], F32, tag="sL")
        sR = pconst.tile([P, 1], F32, tag="sR")
        dq = pconst.tile([P, 1], F32, tag="dq")
        cb = pconst.tile([P, 1], F32, tag="cb")
        nc.vector.memset(cb[:], C_MAGIC)

        if scales is not None:
            # host-computed replicated scales: [:,0]=sL, [:,1]=sR, [:,2]=dq
            sc = pconst.tile([P, 4], F32, tag="sc")
            nc.sync.dma_start(sc[:], scales[:, :])
            nc.vector.tensor_copy(sL[:], sc[:, 0:1])
            nc.vector.tensor_copy(sR[:], sc[:, 1:2])
            nc.vector.tensor_copy(dq[:], sc[:, 2:3])
        else:
            _emit_device_scales(nc, tc, msl, msr, sL, sR, dq, pconst, pred,
                                pstm, pstn)

        # ---------------- quantize into SBUF caches + matmul waves ----------
        # persistent bf16 caches: qn[kt] = full k-row of rhs; qm[kt][c] =
        # 512-col chunk of lhsT (chunk c feeds waves 4c..4c+3)
        qn = [pcache.tile([P, N], BF16, tag=f"qn{kt}", name=f"qn{kt}")
              for kt in range(KT)]
        qm = [[pcache.tile([P, 512], BF16, tag=f"qm{kt}_{c}",
                           name=f"qm{kt}_{c}")
               for c in range(4)] for kt in range(KT)]

        def quant_n(kt, h):
            st = pstn.tile([P, N // 2], F32, tag="stn")
            nc.sync.dma_start(st[:], rhs[kt * P:(kt + 1) * P,
                                         h * (N // 2):(h + 1) * (N // 2)])
            nc.scalar.activation(st[:], st[:], AF.Identity, bias=cb[:],
                                 scale=sR[:])
            nc.vector.tensor_scalar_add(
                qn[kt][:, h * (N // 2):(h + 1) * (N // 2)], st[:], -C_MAGIC)

        def quant_m(kt, c):
            st = pstm.tile([P, 512], F32, tag="stm")
            nc.sync.dma_start(st[:], lhsT[kt * P:(kt + 1) * P,
                                          c * 512:(c + 1) * 512])
            nc.scalar.activation(st[:], st[:], AF.Identity, bias=cb[:],
                                 scale=sL[:])
            nc.vector.tensor_scalar_add(qm[kt][c][:], st[:], -C_MAGIC)

        def wave(mt):
            c, mi = mt // 4, mt % 4
            psums = [ppsum.tile([P, 512], F32, tag="ps", name=f"ps{mt}_{nb}")
                     for nb in range(NB)]
            for kt in range(KT):
                w_ap = qm[kt][c][:, mi * 128:(mi + 1) * 128]
                for nb in range(NB):
                    nc.tensor.matmul(psums[nb][:], w_ap,
                                     qn[kt][:, nb * 512:(nb + 1) * 512],
                                     start=(kt == 0), stop=(kt == KT - 1))
            m0 = mt * P
            for nb in range(NB):
                o = post.tile([P, 512], F32, tag="ost")
                nc.scalar.activation(o[:], psums[nb][:], AF.Copy, scale=dq[:])
                nc.gpsimd.dma_start(out[m0:m0 + P, nb * 512:(nb + 1) * 512],
                                    o[:])

        # emission: group quantize chunks with the waves that first need them
        # so per-engine FIFO order stays pipeline-consistent.
        for kt in range(KT):
            quant_n(kt, 0)
            quant_n(kt, 1)
            quant_m(kt, 0)
        wave(0); wave(1); wave(2); wave(3)
        for g in range(1, 4):
            for kt in range(KT):
                quant_m(kt, g)
            for w in range(4):
                wave(4 * g + w)


def _emit_device_scales(nc, tc, msl, msr, sL, sR, dq, pconst, pred, pstm,
                        pstn):
    """absmax of this core's disjoint slices + cross-core AllReduce(max)."""
    ctx_pool = tc.tile_pool(name="dram", bufs=1, space="DRAM")
    pdram = ctx_pool.__enter__()
    accl = pconst.tile([P, 1], F32, tag="accl")
    accr = pconst.tile([P, 1], F32, tag="accr")
    for kt in range(KT):
        st = pstm.tile([P, MSL_W], F32, tag="stm")
        nc.sync.dma_start(st[:], msl[kt * P:(kt + 1) * P, :])
        if kt == 0:
            nc.vector.reduce_max(accl[:], st[:], axis=mybir.AxisListType.X,
                                 apply_absolute_value=True)
        else:
            r = pred.tile([P, 1], F32, tag="rl")
            nc.vector.reduce_max(r[:], st[:], axis=mybir.AxisListType.X,
                                 apply_absolute_value=True)
            nc.vector.tensor_max(accl[:], accl[:], r[:])
    for kt in range(KT):
        st = pstn.tile([P, MSR_W], F32, tag="stn_p1", name="stn_p1")
        nc.sync.dma_start(st[:], msr[kt * P:(kt + 1) * P, :])
        if kt == 0:
            nc.vector.reduce_max(accr[:], st[:], axis=mybir.AxisListType.X,
                                 apply_absolute_value=True)
        else:
            r = pred.tile([P, 1], F32, tag="rr")
            nc.vector.reduce_max(r[:], st[:], axis=mybir.AxisListType.X,
                                 apply_absolute_value=True)
            nc.vector.tensor_max(accr[:], accr[:], r[:])

    pk = pconst.tile([P, 2], F32, tag="pk")
    nc.vector.tensor_copy(pk[:, 0:1], accl[:])
    nc.vector.tensor_copy(pk[:, 1:2], accr[:])
    gk = pconst.tile([P, 2], F32, tag="gk")
    nc.gpsimd.partition_all_reduce(gk[:], pk[:], channels=P,
                                   reduce_op=bass_isa.ReduceOp.max)

    cc_in = pdram.tile([P, 2], F32, tag="cc_in")
    cc_out = pdram.tile([P, 2], F32, tag="cc_out")
    nc.sync.dma_start(cc_in[:], gk[:])
    nc.gpsimd.collective_compute(
        "AllReduce", mybir.AluOpType.max,
        replica_groups=[list(range(NCORES))],
        ins=[cc_in[:].opt()], outs=[cc_out[:].opt()])
    gsb = pconst.tile([P, 2], F32, tag="gsb")
    nc.sync.dma_start(gsb[:], cc_out[:])

    m2l = pconst.tile([P, 1], F32, tag="m2l")
    m2r = pconst.tile([P, 1], F32, tag="m2r")
    nc.vector.tensor_scalar_max(m2l[:], gsb[:, 0:1], 1e-6)
    nc.vector.tensor_scalar_max(m2r[:], gsb[:, 1:2], 1e-6)
    nc.vector.reciprocal(sL[:], m2l[:])
    nc.vector.tensor_scalar_mul(sL[:], sL[:], CLIP)
    nc.vector.reciprocal(sR[:], m2r[:])
    nc.vector.tensor_scalar_mul(sR[:], sR[:], CLIP)
    nc.vector.tensor_tensor(dq[:], m2l[:], m2r[:], op=mybir.AluOpType.mult)
    nc.vector.tensor_scalar_mul(dq[:], dq[:], 1.0 / (CLIP * CLIP))


_NC_CACHE = {}


def _get_nc(device_scales):
    if device_scales not in _NC_CACHE:
        _NC_CACHE[device_scales] = _build_nc(device_scales)
    return _NC_CACHE[device_scales]


LAST_RESULT = None  # BassKernelResults of the most recent run (for test.py)


def kernel(lhs, rhs, _trace=False, _trace_cores=None,
           _device_scales=DEVICE_SCALES):
    global LAST_RESULT
    lhs = np.ascontiguousarray(np.asarray(lhs, dtype=np.float32))
    rhs = np.ascontiguousarray(np.asarray(rhs, dtype=np.float32))
    assert lhs.shape == (M_FULL, K) and rhs.shape == (K, N_FULL)

    lhsT = np.ascontiguousarray(lhs.T)  # [K, M_FULL]
    if not _device_scales:
        # exact mirror of the reference reduction (order-independent in f32)
        ml = np.maximum(np.abs(lhs).max(), np.float32(1e-6))
        mr = np.maximum(np.abs(rhs).max(), np.float32(1e-6))
        s_l = np.float32(CLIP) / ml
        s_r = np.float32(CLIP) / mr
        d_q = (np.float32(1.0) / s_l) * (np.float32(1.0) / s_r)
        sc = np.tile(np.array([s_l, s_r, d_q, 0.0], dtype=np.float32), (P, 1))

    in_maps = []
    for i in range(RI):
        lT = np.ascontiguousarray(lhsT[:, i * M:(i + 1) * M])
        for j in range(CJ):
            r = np.ascontiguousarray(rhs[:, j * N:(j + 1) * N])
            m = {"lhsT": lT, "rhs": r}
            if _device_scales:
                m["msl"] = np.ascontiguousarray(
                    lT[:, j * MSL_W:(j + 1) * MSL_W])
                m["msr"] = np.ascontiguousarray(
                    r[:, i * MSR_W:(i + 1) * MSR_W])
            else:
                m["scales"] = sc
            in_maps.append(m)

    nc = _get_nc(_device_scales)
    res = run_bass_kernel_spmd(
        nc, in_maps, core_ids=list(range(NCORES)),
        trace=_trace,
        **({"trace_cores": _trace_cores} if _trace_cores else {}))
    LAST_RESULT = res

    full = np.empty((M_FULL, N_FULL), dtype=np.float32)
    for i in range(RI):
        for j in range(CJ):
            full[i * M:(i + 1) * M, j * N:(j + 1) * N] = \
                res.results[i * CJ + j]["out"]
    return full


# revision 8
# speedup vs baseline: 1.0153x; 1.0153x over previous
"""Distributed AQT int8 fake-quant matmul on 8 Trainium2 NeuronCores.

Computes reference:
    lhs_q = fake_quant_int8(lhs); rhs_q = fake_quant_int8(rhs)
    out = lhs_q @ rhs_q            # [4096, 8192] f32

Sharding: 2x4 core grid. Core (i,j) computes the [2048, 2048] output block
(M-half i, N-quarter j) as a K=2048 matmul.

Per the sharding hint, the per-tensor scale is replicated: the global absmax
scale (2 scalars) is computed on host in f32 (bit-identical to the reference
reduction, which is order-independent) and broadcast to all cores; each
device quantizes its shard locally. Set DEVICE_SCALES=True to instead
compute the absmax fully on-device (disjoint 1/8 slices per core + a [128,2]
AllReduce(max) collective) — same numerics to ~1ulp, but pays the ncfw
collective entry barrier (~80us on this runtime).

Quantized values (ints in [-127,127]) are exact in bf16, so the matmul runs
at full bf16 PE rate and the result matches the f32 fake-quant reference to
~1e-6. Rounding uses the magic-constant trick: bf16(f32(x*s + 1.5*2^23) - C)
== round-half-even(x*s), bit-identical to jnp.round.

Pipeline (per core): stream f32 shards once; quantize on ACT (x*s+C) + DVE
(-C, cast bf16) into persistent SBUF caches; 1024 bf16 matmuls in 16 waves
(one 128-row m-tile x full N=2048 per wave, 4 PSUM banks, one weight load
per 4 matmuls); dequantized PSUM evacuation on ACT; outputs DMAed via
gpsimd so input DMAs (sync) are never queued behind them.
"""

import numpy as np

import concourse.bass as bass
import concourse.bass_isa as bass_isa
import concourse.mybir as mybir
import concourse.tile as tile
from concourse import bacc
from concourse.bass_utils import run_bass_kernel_spmd

# Problem shape (hardcoded per contract)
M_FULL, K, N_FULL = 4096, 2048, 8192
RI, CJ = 2, 4                      # core grid: M shards x N shards
M, N = M_FULL // RI, N_FULL // CJ  # 2048 x 2048 per-core output block
P = 128
KT = K // P                        # 16 k-tiles
MT = M // P                        # 16 m-tiles (one wave each)
NB = N // 512                      # 4 n-blocks of 512
MSL_W = M // CJ                    # 512: per-core lhs max-slice width
MSR_W = N // RI                    # 1024: per-core rhs max-slice width
C_MAGIC = 12582912.0               # 1.5 * 2^23
CLIP = 127.0
NCORES = RI * CJ

F32 = mybir.dt.float32
BF16 = mybir.dt.bfloat16
AF = mybir.ActivationFunctionType

DEVICE_SCALES = False  # True: on-device absmax + AllReduce(max) collective

# tuning knobs
STN_BUFS = 8   # f32 input staging (rhs k-row halves)
STM_BUFS = 6   # [P,512] f32 input staging (lhsT chunks)
OST_BUFS = 4   # [P,512] f32 output staging


def _build_nc(device_scales):
    nc = bacc.Bacc("TRN2", target_bir_lowering=False, debug=False,
                   num_devices=NCORES)
    lhsT = nc.dram_tensor("lhsT", [K, M], F32, kind="ExternalInput")
    rhs = nc.dram_tensor("rhs", [K, N], F32, kind="ExternalInput")
    if device_scales:
        msl = nc.dram_tensor("msl", [K, MSL_W], F32, kind="ExternalInput")
        msr = nc.dram_tensor("msr", [K, MSR_W], F32, kind="ExternalInput")
    else:
        msl = msr = None
        scales = nc.dram_tensor("scales", [P, 4], F32, kind="ExternalInput")
    out = nc.dram_tensor("out", [M, N], F32, kind="ExternalOutput")

    with tile.TileContext(nc) as tc:
        if device_scales:
            _emit(nc, tc, lhsT, rhs, out, msl=msl, msr=msr)
        else:
            _emit(nc, tc, lhsT, rhs, out, scales=scales)
    nc.compile()
    return nc


def _emit(nc, tc, lhsT, rhs, out, msl=None, msr=None, scales=None):
    from contextlib import ExitStack
    ctx = ExitStack()
    with ctx:
        pconst = ctx.enter_context(tc.tile_pool(name="const", bufs=1))
        pred = ctx.enter_context(tc.tile_pool(name="red", bufs=3))
        pstn = ctx.enter_context(tc.tile_pool(name="stn", bufs=STN_BUFS))
        pstm = ctx.enter_context(tc.tile_pool(name="stm", bufs=STM_BUFS))
        pcache = ctx.enter_context(tc.tile_pool(name="cache", bufs=1))
        ppsum = ctx.enter_context(tc.tile_pool(name="psum", bufs=8, space="PSUM"))
        post = ctx.enter_context(tc.tile_pool(name="ost", bufs=OST_BUFS))

        sL = pconst.tile([P, 1], F32, tag="sL")
        sR = pconst.tile([P, 1], F32, tag="sR")
        dq = pconst.tile([P, 1], F32, tag="dq")
        cb = pconst.tile([P, 1], F32, tag="cb")
        nc.vector.memset(cb[:], C_MAGIC)

        if scales is not None:
            # host-computed replicated scales: [:,0]=sL, [:,1]=sR, [:,2]=dq
            sc = pconst.tile([P, 4], F32, tag="sc")
            nc.sync.dma_start(sc[:], scales[:, :])
            nc.vector.tensor_copy(sL[:], sc[:, 0:1])
            nc.vector.tensor_copy(sR[:], sc[:, 1:2])
            nc.vector.tensor_copy(dq[:], sc[:, 2:3])
        else:
            _emit_device_scales(nc, tc, msl, msr, sL, sR, dq, pconst, pred,
                                pstm, pstn)

        # ---------------- quantize into SBUF caches + matmul waves ----------
        # persistent bf16 caches: qn[kt] = full k-row of rhs; qm[kt][c] =
        # 512-col chunk of lhsT (chunk c feeds waves 4c..4c+3)
        qn = [pcache.tile([P, N], BF16, tag=f"qn{kt}", name=f"qn{kt}")
              for kt in range(KT)]
        qm = [[pcache.tile([P, 512], BF16, tag=f"qm{kt}_{c}",
                           name=f"qm{kt}_{c}")
               for c in range(4)] for kt in range(KT)]

        def quant_n(kt, h):
            st = pstn.tile([P, N // 2], F32, tag="stn")
            nc.sync.dma_start(st[:], rhs[kt * P:(kt + 1) * P,
                                         h * (N // 2):(h + 1) * (N // 2)])
            nc.scalar.activation(st[:], st[:], AF.Identity, bias=cb[:],
                                 scale=sR[:])
            nc.vector.tensor_scalar_add(
                qn[kt][:, h * (N // 2):(h + 1) * (N // 2)], st[:], -C_MAGIC)

        def quant_m(kt, c):
            st = pstm.tile([P, 512], F32, tag="stm")
            nc.sync.dma_start(st[:], lhsT[kt * P:(kt + 1) * P,
                                          c * 512:(c + 1) * 512])
            nc.scalar.activation(st[:], st[:], AF.Identity, bias=cb[:],
                                 scale=sL[:])
            nc.vector.tensor_scalar_add(qm[kt][c][:], st[:], -C_MAGIC)

        def wave(mt):
            c, mi = mt // 4, mt % 4
            psums = [ppsum.tile([P, 512], F32, tag="ps", name=f"ps{mt}_{nb}")
                     for nb in range(NB)]
            for kt in range(KT):
                w_ap = qm[kt][c][:, mi * 128:(mi + 1) * 128]
                for nb in range(NB):
                    nc.tensor.matmul(psums[nb][:], w_ap,
                                     qn[kt][:, nb * 512:(nb + 1) * 512],
                                     start=(kt == 0), stop=(kt == KT - 1))
            m0 = mt * P
            for nb in range(NB):
                o = post.tile([P, 512], F32, tag="ost")
                nc.scalar.activation(o[:], psums[nb][:], AF.Copy, scale=dq[:])
                nc.gpsimd.dma_start(out[m0:m0 + P, nb * 512:(nb + 1) * 512],
                                    o[:])

        # emission: group quantize chunks with the waves that first need them
        # so per-engine FIFO order stays pipeline-consistent.
        for kt in range(KT):
            quant_n(kt, 0)
            quant_n(kt, 1)
            quant_m(kt, 0)
        wave(0); wave(1); wave(2); wave(3)
        for g in range(1, 4):
            for kt in range(KT):
                quant_m(kt, g)
            for w in range(4):
                wave(4 * g + w)


def _emit_device_scales(nc, tc, msl, msr, sL, sR, dq, pconst, pred, pstm,
                        pstn):
    """absmax of this core's disjoint slices + cross-core AllReduce(max)."""
    ctx_pool = tc.tile_pool(name="dram", bufs=1, space="DRAM")
    pdram = ctx_pool.__enter__()
    accl = pconst.tile([P, 1], F32, tag="accl")
    accr = pconst.tile([P, 1], F32, tag="accr")
    for kt in range(KT):
        st = pstm.tile([P, MSL_W], F32, tag="stm")
        nc.sync.dma_start(st[:], msl[kt * P:(kt + 1) * P, :])
        if kt == 0:
            nc.vector.reduce_max(accl[:], st[:], axis=mybir.AxisListType.X,
                                 apply_absolute_value=True)
        else:
            r = pred.tile([P, 1], F32, tag="rl")
            nc.vector.reduce_max(r[:], st[:], axis=mybir.AxisListType.X,
                                 apply_absolute_value=True)
            nc.vector.tensor_max(accl[:], accl[:], r[:])
    for kt in range(KT):
        st = pstn.tile([P, MSR_W], F32, tag="stn_p1", name="stn_p1")
        nc.sync.dma_start(st[:], msr[kt * P:(kt + 1) * P, :])
        if kt == 0:
            nc.vector.reduce_max(accr[:], st[:], axis=mybir.AxisListType.X,
                                 apply_absolute_value=True)
        else:
            r = pred.tile([P, 1], F32, tag="rr")
            nc.vector.reduce_max(r[:], st[:], axis=mybir.AxisListType.X,
                                 apply_absolute_value=True)
            nc.vector.tensor_max(accr[:], accr[:], r[:])

    pk = pconst.tile([P, 2], F32, tag="pk")
    nc.vector.tensor_copy(pk[:, 0:1], accl[:])
    nc.vector.tensor_copy(pk[:, 1:2], accr[:])
    gk = pconst.tile([P, 2], F32, tag="gk")
    nc.gpsimd.partition_all_reduce(gk[:], pk[:], channels=P,
                                   reduce_op=bass_isa.ReduceOp.max)

    cc_in = pdram.tile([P, 2], F32, tag="cc_in")
    cc_out = pdram.tile([P, 2], F32, tag="cc_out")
    nc.sync.dma_start(cc_in[:], gk[:])
    nc.gpsimd.collective_compute(
        "AllReduce", mybir.AluOpType.max,
        replica_groups=[list(range(NCORES))],
        ins=[cc_in[:].opt()], outs=[cc_out[:].opt()])
    gsb = pconst.tile([P, 2], F32, tag="gsb")
    nc.sync.dma_start(gsb[:], cc_out[:])

    m2l = pconst.tile([P, 1], F32, tag="m2l")
    m2r = pconst.tile([P, 1], F32, tag="m2r")
    nc.vector.tensor_scalar_max(m2l[:], gsb[:, 0:1], 1e-6)
    nc.vector.tensor_scalar_max(m2r[:], gsb[:, 1:2], 1e-6)
    nc.vector.reciprocal(sL[:], m2l[:])
    nc.vector.tensor_scalar_mul(sL[:], sL[:], CLIP)
    nc.vector.reciprocal(sR[:], m2r[:])
    nc.vector.tensor_scalar_mul(sR[:], sR[:], CLIP)
    nc.vector.tensor_tensor(dq[:], m2l[:], m2r[:], op=mybir.AluOpType.mult)
    nc.vector.tensor_scalar_mul(dq[:], dq[:], 1.0 / (CLIP * CLIP))


_NC_CACHE = {}


def _get_nc(device_scales):
    if device_scales not in _NC_CACHE:
        _NC_CACHE[device_scales] = _build_nc(device_scales)
    return _NC_CACHE[device_scales]


LAST_RESULT = None  # BassKernelResults of the most recent run (for test.py)


def kernel(lhs, rhs, _trace=False, _trace_cores=None,
           _device_scales=DEVICE_SCALES):
    global LAST_RESULT
    lhs = np.ascontiguousarray(np.asarray(lhs, dtype=np.float32))
    rhs = np.ascontiguousarray(np.asarray(rhs, dtype=np.float32))
    assert lhs.shape == (M_FULL, K) and rhs.shape == (K, N_FULL)

    lhsT = np.ascontiguousarray(lhs.T)  # [K, M_FULL]
    if not _device_scales:
        # exact mirror of the reference reduction (order-independent in f32)
        ml = np.maximum(np.abs(lhs).max(), np.float32(1e-6))
        mr = np.maximum(np.abs(rhs).max(), np.float32(1e-6))
        s_l = np.float32(CLIP) / ml
        s_r = np.float32(CLIP) / mr
        d_q = (np.float32(1.0) / s_l) * (np.float32(1.0) / s_r)
        sc = np.tile(np.array([s_l, s_r, d_q, 0.0], dtype=np.float32), (P, 1))

    in_maps = []
    for i in range(RI):
        lT = np.ascontiguousarray(lhsT[:, i * M:(i + 1) * M])
        for j in range(CJ):
            r = np.ascontiguousarray(rhs[:, j * N:(j + 1) * N])
            m = {"lhsT": lT, "rhs": r}
            if _device_scales:
                m["msl"] = np.ascontiguousarray(
                    lT[:, j * MSL_W:(j + 1) * MSL_W])
                m["msr"] = np.ascontiguousarray(
                    r[:, i * MSR_W:(i + 1) * MSR_W])
            else:
                m["scales"] = sc
            in_maps.append(m)

    nc = _get_nc(_device_scales)
    res = run_bass_kernel_spmd(
        nc, in_maps, core_ids=list(range(NCORES)),
        trace=_trace,
        **({"trace_cores": _trace_cores} if _trace_cores else {}))
    LAST_RESULT = res

    full = np.empty((M_FULL, N_FULL), dtype=np.float32)
    for i in range(RI):
        for j in range(CJ):
            full[i * M:(i + 1) * M, j * N:(j + 1) * N] = \
                res.results[i * CJ + j]["out"]
    return full


# revision 9
# speedup vs baseline: 1.0379x; 1.0223x over previous
"""Distributed AQT int8 fake-quant matmul on 8 Trainium2 NeuronCores.

Computes reference:
    lhs_q = fake_quant_int8(lhs); rhs_q = fake_quant_int8(rhs)
    out = lhs_q @ rhs_q            # [4096, 8192] f32

Sharding: 2x4 core grid. Core (i,j) computes the [2048, 2048] output block
(M-half i, N-quarter j) as a K=2048 matmul.

Per the sharding hint, the per-tensor scale is replicated: the global absmax
scale (2 scalars) is computed on host in f32 (bit-identical to the reference
reduction, which is order-independent) and broadcast to all cores; each
device quantizes its shard locally. Set DEVICE_SCALES=True to instead
compute the absmax fully on-device (disjoint 1/8 slices per core + a [128,2]
AllReduce(max) collective) — same numerics to ~1ulp, but pays the ncfw
collective entry barrier (~80us on this runtime).

Quantized values (ints in [-127,127]) are exact in bf16, so the matmul runs
at full bf16 PE rate and the result matches the f32 fake-quant reference to
~1e-6. Rounding uses the magic-constant trick: bf16(f32(x*s + 1.5*2^23) - C)
== round-half-even(x*s), bit-identical to jnp.round.

Pipeline (per core): stream f32 shards once; quantize on ACT (x*s+C) + DVE
(-C, cast bf16) into persistent SBUF caches; 1024 bf16 matmuls in 16 waves
(one 128-row m-tile x full N=2048 per wave, 4 PSUM banks, one weight load
per 4 matmuls); dequantized PSUM evacuation on ACT; outputs DMAed via
gpsimd so input DMAs (sync) are never queued behind them.
"""

import numpy as np

import concourse.bass as bass
import concourse.bass_isa as bass_isa
import concourse.mybir as mybir
import concourse.tile as tile
from concourse import bacc
from concourse.bass_utils import run_bass_kernel_spmd

# Problem shape (hardcoded per contract)
M_FULL, K, N_FULL = 4096, 2048, 8192
RI, CJ = 2, 4                      # core grid: M shards x N shards
M, N = M_FULL // RI, N_FULL // CJ  # 2048 x 2048 per-core output block
P = 128
KT = K // P                        # 16 k-tiles
MT = M // P                        # 16 m-tiles (one wave each)
NB = N // 512                      # 4 n-blocks of 512
MSL_W = M // CJ                    # 512: per-core lhs max-slice width
MSR_W = N // RI                    # 1024: per-core rhs max-slice width
C_MAGIC = 12582912.0               # 1.5 * 2^23
CLIP = 127.0
NCORES = RI * CJ

F32 = mybir.dt.float32
BF16 = mybir.dt.bfloat16
AF = mybir.ActivationFunctionType

DEVICE_SCALES = False  # True: on-device absmax + AllReduce(max) collective

# tuning knobs
STN_BUFS = 4   # [P,2048] f32 input staging (rhs k-rows)
STM_BUFS = 6   # [P,512] f32 input staging (lhsT chunks)
OST_BUFS = 4   # [P,512] f32 output staging


def _build_nc(device_scales):
    nc = bacc.Bacc("TRN2", target_bir_lowering=False, debug=False,
                   num_devices=NCORES)
    lhsT = nc.dram_tensor("lhsT", [K, M], F32, kind="ExternalInput")
    rhs = nc.dram_tensor("rhs", [K, N], F32, kind="ExternalInput")
    if device_scales:
        msl = nc.dram_tensor("msl", [K, MSL_W], F32, kind="ExternalInput")
        msr = nc.dram_tensor("msr", [K, MSR_W], F32, kind="ExternalInput")
    else:
        msl = msr = None
        scales = nc.dram_tensor("scales", [P, 4], F32, kind="ExternalInput")
    out = nc.dram_tensor("out", [M, N], F32, kind="ExternalOutput")

    with tile.TileContext(nc) as tc:
        if device_scales:
            _emit(nc, tc, lhsT, rhs, out, msl=msl, msr=msr)
        else:
            _emit(nc, tc, lhsT, rhs, out, scales=scales)
    nc.compile()
    return nc


def _emit(nc, tc, lhsT, rhs, out, msl=None, msr=None, scales=None):
    from contextlib import ExitStack
    ctx = ExitStack()
    with ctx:
        pconst = ctx.enter_context(tc.tile_pool(name="const", bufs=1))
        pred = ctx.enter_context(tc.tile_pool(name="red", bufs=3))
        pstn = ctx.enter_context(tc.tile_pool(name="stn", bufs=STN_BUFS))
        pstm = ctx.enter_context(tc.tile_pool(name="stm", bufs=STM_BUFS))
        pcache = ctx.enter_context(tc.tile_pool(name="cache", bufs=1))
        ppsum = ctx.enter_context(tc.tile_pool(name="psum", bufs=8, space="PSUM"))
        post = ctx.enter_context(tc.tile_pool(name="ost", bufs=OST_BUFS))

        sL = pconst.tile([P, 1], F32, tag="sL")
        sR = pconst.tile([P, 1], F32, tag="sR")
        dq = pconst.tile([P, 1], F32, tag="dq")
        cb = pconst.tile([P, 1], F32, tag="cb")
        nc.vector.memset(cb[:], C_MAGIC)

        if scales is not None:
            # host-computed replicated scales: [:,0]=sL, [:,1]=sR, [:,2]=dq
            sc = pconst.tile([P, 4], F32, tag="sc")
            nc.sync.dma_start(sc[:], scales[:, :])
            nc.vector.tensor_copy(sL[:], sc[:, 0:1])
            nc.vector.tensor_copy(sR[:], sc[:, 1:2])
            nc.vector.tensor_copy(dq[:], sc[:, 2:3])
        else:
            _emit_device_scales(nc, tc, msl, msr, sL, sR, dq, pconst, pred,
                                pstm, pstn)

        # ---------------- quantize into SBUF caches + matmul waves ----------
        # persistent bf16 caches: qn[kt] = full k-row of rhs; qm[kt][c] =
        # 512-col chunk of lhsT (chunk c feeds waves 4c..4c+3)
        qn = [pcache.tile([P, N], BF16, tag=f"qn{kt}", name=f"qn{kt}")
              for kt in range(KT)]
        qm = [[pcache.tile([P, 512], BF16, tag=f"qm{kt}_{c}",
                           name=f"qm{kt}_{c}")
               for c in range(4)] for kt in range(KT)]

        def quant_n(kt):
            st = pstn.tile([P, N], F32, tag="stn")
            nc.sync.dma_start(st[:], rhs[kt * P:(kt + 1) * P, :])
            nc.scalar.activation(st[:], st[:], AF.Identity, bias=cb[:],
                                 scale=sR[:])
            nc.vector.tensor_scalar_add(qn[kt][:], st[:], -C_MAGIC)

        def quant_m(kt, c):
            st = pstm.tile([P, 512], F32, tag="stm")
            nc.sync.dma_start(st[:], lhsT[kt * P:(kt + 1) * P,
                                          c * 512:(c + 1) * 512])
            nc.scalar.activation(st[:], st[:], AF.Identity, bias=cb[:],
                                 scale=sL[:])
            nc.vector.tensor_scalar_add(qm[kt][c][:], st[:], -C_MAGIC)

        def wave(mt):
            c, mi = mt // 4, mt % 4
            psums = [ppsum.tile([P, 512], F32, tag="ps", name=f"ps{mt}_{nb}")
                     for nb in range(NB)]
            for kt in range(KT):
                w_ap = qm[kt][c][:, mi * 128:(mi + 1) * 128]
                for nb in range(NB):
                    nc.tensor.matmul(psums[nb][:], w_ap,
                                     qn[kt][:, nb * 512:(nb + 1) * 512],
                                     start=(kt == 0), stop=(kt == KT - 1))
            m0 = mt * P
            for nb in range(NB):
                o = post.tile([P, 512], F32, tag="ost")
                nc.scalar.activation(o[:], psums[nb][:], AF.Copy, scale=dq[:])
                nc.gpsimd.dma_start(out[m0:m0 + P, nb * 512:(nb + 1) * 512],
                                    o[:])

        # emission: group quantize chunks with the waves that first need them
        # so per-engine FIFO order stays pipeline-consistent.
        for kt in range(KT):
            quant_n(kt)
            quant_m(kt, 0)
        wave(0); wave(1); wave(2); wave(3)
        for g in range(1, 4):
            for kt in range(KT):
                quant_m(kt, g)
            for w in range(4):
                wave(4 * g + w)


def _emit_device_scales(nc, tc, msl, msr, sL, sR, dq, pconst, pred, pstm,
                        pstn):
    """absmax of this core's disjoint slices + cross-core AllReduce(max)."""
    ctx_pool = tc.tile_pool(name="dram", bufs=1, space="DRAM")
    pdram = ctx_pool.__enter__()
    accl = pconst.tile([P, 1], F32, tag="accl")
    accr = pconst.tile([P, 1], F32, tag="accr")
    for kt in range(KT):
        st = pstm.tile([P, MSL_W], F32, tag="stm")
        nc.sync.dma_start(st[:], msl[kt * P:(kt + 1) * P, :])
        if kt == 0:
            nc.vector.reduce_max(accl[:], st[:], axis=mybir.AxisListType.X,
                                 apply_absolute_value=True)
        else:
            r = pred.tile([P, 1], F32, tag="rl")
            nc.vector.reduce_max(r[:], st[:], axis=mybir.AxisListType.X,
                                 apply_absolute_value=True)
            nc.vector.tensor_max(accl[:], accl[:], r[:])
    for kt in range(KT):
        st = pstn.tile([P, MSR_W], F32, tag="stn_p1", name="stn_p1")
        nc.sync.dma_start(st[:], msr[kt * P:(kt + 1) * P, :])
        if kt == 0:
            nc.vector.reduce_max(accr[:], st[:], axis=mybir.AxisListType.X,
                                 apply_absolute_value=True)
        else:
            r = pred.tile([P, 1], F32, tag="rr")
            nc.vector.reduce_max(r[:], st[:], axis=mybir.AxisListType.X,
                                 apply_absolute_value=True)
            nc.vector.tensor_max(accr[:], accr[:], r[:])

    pk = pconst.tile([P, 2], F32, tag="pk")
    nc.vector.tensor_copy(pk[:, 0:1], accl[:])
    nc.vector.tensor_copy(pk[:, 1:2], accr[:])
    gk = pconst.tile([P, 2], F32, tag="gk")
    nc.gpsimd.partition_all_reduce(gk[:], pk[:], channels=P,
                                   reduce_op=bass_isa.ReduceOp.max)

    cc_in = pdram.tile([P, 2], F32, tag="cc_in")
    cc_out = pdram.tile([P, 2], F32, tag="cc_out")
    nc.sync.dma_start(cc_in[:], gk[:])
    nc.gpsimd.collective_compute(
        "AllReduce", mybir.AluOpType.max,
        replica_groups=[list(range(NCORES))],
        ins=[cc_in[:].opt()], outs=[cc_out[:].opt()])
    gsb = pconst.tile([P, 2], F32, tag="gsb")
    nc.sync.dma_start(gsb[:], cc_out[:])

    m2l = pconst.tile([P, 1], F32, tag="m2l")
    m2r = pconst.tile([P, 1], F32, tag="m2r")
    nc.vector.tensor_scalar_max(m2l[:], gsb[:, 0:1], 1e-6)
    nc.vector.tensor_scalar_max(m2r[:], gsb[:, 1:2], 1e-6)
    nc.vector.reciprocal(sL[:], m2l[:])
    nc.vector.tensor_scalar_mul(sL[:], sL[:], CLIP)
    nc.vector.reciprocal(sR[:], m2r[:])
    nc.vector.tensor_scalar_mul(sR[:], sR[:], CLIP)
    nc.vector.tensor_tensor(dq[:], m2l[:], m2r[:], op=mybir.AluOpType.mult)
    nc.vector.tensor_scalar_mul(dq[:], dq[:], 1.0 / (CLIP * CLIP))


_NC_CACHE = {}


def _get_nc(device_scales):
    if device_scales not in _NC_CACHE:
        _NC_CACHE[device_scales] = _build_nc(device_scales)
    return _NC_CACHE[device_scales]


LAST_RESULT = None  # BassKernelResults of the most recent run (for test.py)


def kernel(lhs, rhs, _trace=False, _trace_cores=None,
           _device_scales=DEVICE_SCALES):
    global LAST_RESULT
    lhs = np.ascontiguousarray(np.asarray(lhs, dtype=np.float32))
    rhs = np.ascontiguousarray(np.asarray(rhs, dtype=np.float32))
    assert lhs.shape == (M_FULL, K) and rhs.shape == (K, N_FULL)

    lhsT = np.ascontiguousarray(lhs.T)  # [K, M_FULL]
    if not _device_scales:
        # exact mirror of the reference reduction (order-independent in f32)
        ml = np.maximum(np.abs(lhs).max(), np.float32(1e-6))
        mr = np.maximum(np.abs(rhs).max(), np.float32(1e-6))
        s_l = np.float32(CLIP) / ml
        s_r = np.float32(CLIP) / mr
        d_q = (np.float32(1.0) / s_l) * (np.float32(1.0) / s_r)
        sc = np.tile(np.array([s_l, s_r, d_q, 0.0], dtype=np.float32), (P, 1))

    in_maps = []
    for i in range(RI):
        lT = np.ascontiguousarray(lhsT[:, i * M:(i + 1) * M])
        for j in range(CJ):
            r = np.ascontiguousarray(rhs[:, j * N:(j + 1) * N])
            m = {"lhsT": lT, "rhs": r}
            if _device_scales:
                m["msl"] = np.ascontiguousarray(
                    lT[:, j * MSL_W:(j + 1) * MSL_W])
                m["msr"] = np.ascontiguousarray(
                    r[:, i * MSR_W:(i + 1) * MSR_W])
            else:
                m["scales"] = sc
            in_maps.append(m)

    nc = _get_nc(_device_scales)
    res = run_bass_kernel_spmd(
        nc, in_maps, core_ids=list(range(NCORES)),
        trace=_trace,
        **({"trace_cores": _trace_cores} if _trace_cores else {}))
    LAST_RESULT = res

    full = np.empty((M_FULL, N_FULL), dtype=np.float32)
    for i in range(RI):
        for j in range(CJ):
            full[i * M:(i + 1) * M, j * N:(j + 1) * N] = \
                res.results[i * CJ + j]["out"]
    return full
